# revision 72
# baseline (speedup 1.0000x reference)
"""Bahdanau additive attention on TRN2 (Bass/Tile), 8-core data-parallel.

Math (per batch row b):
    qp   = query @ Wq.T                      # [H]
    kp_s = keys[s] @ Wk.T                    # [S, H]
    e_s  = v . tanh(qp + kp_s)               # [S]
    attn = softmax(mask(e))                  # [S]  (all-pad rows -> 0)
    ctx  = attn @ keys                       # [K]

Sharding: B=64 batches split 8 ways (8 per core); weights replicated.

Per-core kernel layout strategy:
  - The projection contracts over k, so keys tiles are PE-transposed
    ([s,k] -> [k,s]) and the matmul runs with Wk^T chunks stationary,
    producing kp^T [h, s] in PSUM (fp32r matmuls: full-rate with fp32
    storage).
  - tanh(kp^T + qp^T) is fused on ScalarE via activation bias (per
    partition = per h).
  - energy = v . tanh(...) is a second PE matmul contracting h.
  - softmax per batch row runs on partition 0 (tiny: [1, S]).
  - context re-streams keys in natural [s, k] layout and contracts s
    with the attention column as the stationary operand.
"""

import os
import sys

import numpy as np

for _p in ("/opt/trn_rl_repo", os.path.expanduser("~/.axon_site/_ro/trn_rl_repo")):
    if os.path.isdir(_p) and _p not in sys.path:
        sys.path.append(_p)

from contextlib import ExitStack

import concourse.bacc as bacc
import concourse.bass as bass
import concourse.mybir as mybir
import concourse.tile as tile
from concourse import bass_isa, bass_utils
from concourse.masks import make_identity

F32 = mybir.dt.float32
F32R = mybir.dt.float32r
BF16 = mybir.dt.bfloat16
U8 = mybir.dt.uint8
P = 128

N_CORES = 8
FULL_B, FULL_S, FULL_H, FULL_K = 64, 2048, 1024, 1024

NEG_BIG = -30.0  # masked-energy fill; |energy| << 30 so never collides


def _emit(ctx, tc, io, B, S, K, H, SB):
    nc = tc.nc
    KC, HC = K // P, H // P
    NB, JB, SC = SB // P, S // SB, S // P
    CTX_CH = [(o, min(512, K - o)) for o in range(0, K, 512)]
    TANH = mybir.ActivationFunctionType.Tanh
    EXP = mybir.ActivationFunctionType.Exp

    keys = io["keys"].ap()
    mask = io["mask"].ap()
    wqT_in = io["wqT"].ap()
    wkT_in = io["wkT"].ap()
    qT_in = io["qT"].ap()
    vT_in = io["vT"].ap()
    ctx_out = io["context"].ap()
    attn_out = io["attn"].ap()

    const = ctx.enter_context(tc.tile_pool(name="const", bufs=1))
    qpT = const.tile([P, HC, B], F32, name="qpT")

    # Host pre-packed weights: wkT/wqT are [P, KC, H] bf16 (W^T with the
    # contraction dim on partitions), qT is [P, KC, B] bf16, vT [P, HC]
    # bf16. Loaded with plain contiguous HWDGE DMAs; no on-chip transpose.
    # DMAs for these are emitted by load_weights() after the first keys
    # block so the kT pipeline owns the head of the sync queue.
    wkT = const.tile([P, KC, H], BF16, name="wkT")
    wqT = const.tile([P, KC, H], BF16, name="wqT")
    qT = const.tile([P, KC, B], BF16, name="qT")
    vTb = const.tile([P, HC], BF16, name="vTb")
    ident1 = const.tile([1, 1], F32, name="ident1")
    neg_big = const.tile([1, SB], F32, name="neg_big")

    def load_weights():
        nc.sync.dma_start(out=wkT, in_=wkT_in)
        nc.sync.dma_start(out=qT, in_=qT_in)
        nc.sync.dma_start(out=wqT, in_=wqT_in)
        nc.sync.dma_start(out=vTb, in_=vT_in)
        nc.vector.memset(ident1, 1.0)
        nc.vector.memset(neg_big, NEG_BIG)

    def wk_lhsT(hc, kc):
        return wkT[:, kc, hc * P : (hc + 1) * P]

    def qp_emit():
        for hc in range(HC):
            psqp = ps_kp.tile([P, B], F32, name="psqp", tag="kp")
            for kc in range(KC):
                nc.tensor.matmul(
                    psqp,
                    wqT[:, kc, hc * P : (hc + 1) * P],
                    qT[:, kc, :],
                    start=(kc == 0),
                    stop=(kc == KC - 1),
                )
            nc.vector.tensor_copy(qpT[:, hc, :], psqp)

    # ---------- main pools ----------
    knp = ctx.enter_context(tc.tile_pool(name="knp", bufs=2))
    knbp = ctx.enter_context(tc.tile_pool(name="knbp", bufs=7))
    ktp = ctx.enter_context(tc.tile_pool(name="ktp", bufs=4))
    thp = ctx.enter_context(tc.tile_pool(name="thp", bufs=9))
    stg = ctx.enter_context(tc.tile_pool(name="stg", bufs=2))
    atp = ctx.enter_context(tc.tile_pool(name="atp", bufs=2))
    PSUM = bass.MemorySpace.PSUM
    ps_kp = ctx.enter_context(tc.tile_pool(name="ps_kp", bufs=3, space=PSUM))
    ps_e = ctx.enter_context(tc.tile_pool(name="ps_e", bufs=2, space=PSUM))
    ps_c = ctx.enter_context(tc.tile_pool(name="ps_c", bufs=2, space=PSUM))

    def start_block(b, j):
        """HWDGE fp32 load -> DVE cast to bf16 -> x-bar DMA transpose."""
        kn = knp.tile([P, NB, K], F32, name="kn", tag="kn")
        nc.sync.dma_start(
            out=kn,
            in_=keys[b, j * SB : (j + 1) * SB, :].rearrange(
                "(nb p) k -> p nb k", p=P
            ),
        )
        knb = knbp.tile([P, NB, K], BF16, name="knb", tag="knb")
        nc.vector.tensor_copy(knb, kn)
        kT = ktp.tile([P, NB, KC, P], BF16, name="kT", tag="kT")
        nc.sync.dma_start_transpose(out=kT, in_=knb)
        return kT, knb

    # Energy matmuls are M=1; pack 4 of them into one PE pass on disjoint
    # 32-col groups (tile_position), giving 4 partial rows at partitions
    # 0/32/64/96 that GpSimd later all-reduces.
    EG = min(4, HC)  # energy col-pack width
    ER = (HC + EG - 1) // EG  # accumulation rounds per position

    def energy_pack(pe4, ths, r):
        for jj, (th, hc) in enumerate(ths):
            nc.tensor.matmul(
                pe4[32 * jj : 32 * jj + 1, :],
                vTb[:, hc : hc + 1],
                th,
                start=(r == 0),
                stop=(r == ER - 1),
                skip_group_check=True,
                tile_position=(0, 32 * jj),
            )

    def evac_energy(pe4, e_row4, b, j):
        sl = e_row4[0:1, j * SB : (j + 1) * SB]
        nc.scalar.copy(sl, pe4[0:1, :])
        for jj in range(1, EG):
            nc.vector.tensor_add(sl, sl, pe4[32 * jj : 32 * jj + 1, :])

    def compute_block(b, j, kT, e_row4, carry, post_first_group=None):
        pe4 = ps_e.tile([P, SB], F32, name="pe4", tag="e")
        ths = []
        r = 0
        for hc in range(HC):
            pk = ps_kp.tile([P, SB], F32, name="pk", tag="kp")
            for kc in range(KC):
                nc.tensor.matmul(
                    pk,
                    wk_lhsT(hc, kc),
                    kT[:, :, kc, :],
                    start=(kc == 0),
                    stop=(kc == KC - 1),
                )
            if post_first_group is not None:
                post_first_group()
                post_first_group = None
            th = thp.tile([P, SB], BF16, name="th", tag="th")
            nc.scalar.activation(th, pk, TANH, bias=qpT[:, hc, b : b + 1], scale=1.0)
            ths.append((th, hc))
            if carry is not None:
                # flush the previous packed group while tanh of this group
                # is still in flight on ScalarE
                flush_carry(carry)
                carry = None
            if len(ths) == EG:
                carry = (pe4, ths, r, (e_row4, b, j))
                ths = []
                r += 1
        return carry

    def flush_carry(carry):
        pe4, ths, r, evac_args = carry
        energy_pack(pe4, ths, r)
        if r == ER - 1:
            evac_energy(pe4, *evac_args)

    def softmax(b, e_row4, m_row):
        e_row = e_row4[0:1, :]
        for j in range(JB):
            nc.vector.copy_predicated(
                e_row[0:1, j * SB : (j + 1) * SB],
                m_row[0:1, j * SB : (j + 1) * SB],
                neg_big,
            )
        mx = stg.tile([1, 1], F32, name="mx", tag="s1")
        nc.vector.reduce_max(out=mx, in_=e_row, axis=mybir.AxisListType.X)
        nm = stg.tile([1, 1], F32, name="nm", tag="s2")
        nc.scalar.mul(nm, mx, -1.0)
        se = stg.tile([1, 1], F32, name="se", tag="s3")
        nc.scalar.activation(e_row, e_row, EXP, bias=nm, scale=1.0, accum_out=se)
        ri = stg.tile([1, 1], F32, name="ri", tag="s4")
        nc.vector.reciprocal(ri, se)
        # all-pad rows: max == NEG_BIG exactly -> zero the whole row
        pad = stg.tile([1, 1], F32, name="pad", tag="s5")
        nc.vector.tensor_scalar(
            out=pad, in0=nm, scalar1=-NEG_BIG, scalar2=None,
            op0=mybir.AluOpType.is_equal,
        )
        valid = stg.tile([1, 1], F32, name="valid", tag="s6")
        nc.vector.tensor_scalar(
            out=valid, in0=pad, scalar1=-1.0, scalar2=1.0,
            op0=mybir.AluOpType.mult, op1=mybir.AluOpType.add,
        )
        ri2 = stg.tile([1, 1], F32, name="ri2", tag="s7")
        nc.vector.tensor_mul(ri2, ri, valid)
        nc.vector.tensor_scalar_mul(e_row, e_row, ri2)
        nc.sync.dma_start(out=attn_out[b : b + 1, :], in_=e_row)
        return e_row

    def attn_transpose(b, a_row):
        at_ps = ps_e.tile([P, SC], F32, name="at_ps", tag="e")
        for sc in range(SC):
            nc.tensor.transpose(
                at_ps[:, sc : sc + 1],
                a_row[0:1, sc * P : (sc + 1) * P],
                ident1,
            )
        atT = atp.tile([P, SC], BF16, name="atT", tag="atT")
        nc.vector.tensor_copy(atT, at_ps)
        return atT

    CG = min(4, SC)  # context col-pack width
    CR = SC // CG

    def context_pass(b, atT, knbs_b):
        pc4s = [
            ps_c.tile([P, chn], F32, name=f"pc4_{i}", tag="c")
            for i, (o, chn) in enumerate(CTX_CH)
        ]
        for rr in range(CR):
            for i, (o, chn) in enumerate(CTX_CH):
                for jj in range(CG):
                    sc = rr * CG + jj
                    j2, nb = divmod(sc, NB)
                    nc.tensor.matmul(
                        pc4s[i][32 * jj : 32 * jj + 1, :],
                        atT[:, sc : sc + 1],
                        knbs_b[j2][:, nb, o : o + chn],
                        start=(rr == 0),
                        stop=(rr == CR - 1),
                        skip_group_check=True,
                        tile_position=(0, 32 * jj),
                    )
        ctx4 = stg.tile([1, K], F32, name="ctx4", tag="crow")
        for i, (o, chn) in enumerate(CTX_CH):
            sl = ctx4[0:1, o : o + chn]
            nc.scalar.copy(sl, pc4s[i][0:1, :])
            for jj in range(1, CG):
                nc.vector.tensor_add(sl, sl, pc4s[i][32 * jj : 32 * jj + 1, :])
        nc.sync.dma_start(out=ctx_out[b : b + 1, :], in_=ctx4[0:1, :])

    # ---------- main schedule ----------
    def new_b_tiles(b):
        e = stg.tile([1, S], F32, name="e_row4", tag="erow")
        m = stg.tile([1, S], U8, name="m_row", tag="mrow")
        nc.sync.dma_start(out=m, in_=mask[b : b + 1, :])
        return e, m

    blocks = [(b, j) for b in range(B) for j in range(JB)]
    LOOKAHEAD = 2
    e_rows, m_rows, a_rows = {}, {}, {}
    knbs, kts = {}, {}
    e_rows[0], m_rows[0] = new_b_tiles(0)
    kts[blocks[0]], knbs[blocks[0]] = start_block(*blocks[0])
    load_weights()
    for i in range(1, min(LOOKAHEAD, len(blocks))):
        kts[blocks[i]], knbs[blocks[i]] = start_block(*blocks[i])
    carry = None
    for idx, (b, j) in enumerate(blocks):
        if idx + LOOKAHEAD < len(blocks):
            nxt_blk = blocks[idx + LOOKAHEAD]
            kts[nxt_blk], knbs[nxt_blk] = start_block(*nxt_blk)
        carry = compute_block(
            b, j, kts.pop((b, j)), e_rows[b], carry,
            post_first_group=qp_emit if idx == 0 else None,
        )
        if j == 0 and b > 0:
            context_pass(
                b - 1,
                attn_transpose(b - 1, a_rows[b - 1]),
                [knbs.pop((b - 1, jj)) for jj in range(JB)],
            )
        if j == JB - 1:
            if carry is not None:
                flush_carry(carry)
                carry = None
            a_rows[b] = softmax(b, e_rows[b], m_rows[b])
            if b + 1 < B:
                e_rows[b + 1], m_rows[b + 1] = new_b_tiles(b + 1)
    context_pass(
        B - 1,
        attn_transpose(B - 1, a_rows[B - 1]),
        [knbs.pop((B - 1, jj)) for jj in range(JB)],
    )


def build_kernel(B=FULL_B // N_CORES, S=FULL_S, K=FULL_K, H=FULL_H, SB=512):
    KC, HC = K // P, H // P
    nc = bacc.Bacc(
        "TRN2", target_bir_lowering=False, debug=False, enable_partition_id=False
    )
    io = {
        "keys": nc.dram_tensor("keys", [B, S, K], F32, kind="ExternalInput"),
        "mask": nc.dram_tensor("mask", [B, S], U8, kind="ExternalInput"),
        "wqT": nc.dram_tensor("wqT", [P, KC, H], BF16, kind="ExternalInput"),
        "wkT": nc.dram_tensor("wkT", [P, KC, H], BF16, kind="ExternalInput"),
        "qT": nc.dram_tensor("qT", [P, KC, B], BF16, kind="ExternalInput"),
        "vT": nc.dram_tensor("vT", [P, HC], BF16, kind="ExternalInput"),
        "context": nc.dram_tensor("context", [B, K], F32, kind="ExternalOutput"),
        "attn": nc.dram_tensor("attn", [B, S], F32, kind="ExternalOutput"),
    }
    with tile.TileContext(nc) as tc:
        with ExitStack() as ctx:
            _emit(ctx, tc, io, B, S, K, H, SB)
    nc.compile()
    return nc


def pack_weights(Wq, Wk, v, query):
    """Host-side re-layout: W^T/q^T/v^T in bf16 with the contraction dim
    on partitions. Pure marshalling (transpose + dtype cast), no math."""
    import ml_dtypes

    bf16 = ml_dtypes.bfloat16
    H, K = Wk.shape
    KC, HC = K // P, H // P
    Bq = query.shape[0]
    # wT[p, kc, h] = W[h, kc*P + p]
    wqT = np.ascontiguousarray(
        Wq.T.reshape(KC, P, H).transpose(1, 0, 2).astype(bf16)
    )
    wkT = np.ascontiguousarray(
        Wk.T.reshape(KC, P, H).transpose(1, 0, 2).astype(bf16)
    )
    # qT[p, kc, b] = query[b, kc*P + p]
    qT = np.ascontiguousarray(
        query.reshape(Bq, KC, P).transpose(2, 1, 0).astype(bf16)
    )
    # vT[p, hc] = v[0, hc*P + p]
    vT = np.ascontiguousarray(v[0].reshape(HC, P).T.astype(bf16))
    return wqT, wkT, qT, vT


def ref_np(query, keys, mask, Wq, Wk, v):
    """Numpy reference (mirrors the jax oracle) for dev testing."""
    qp = query.astype(np.float64) @ Wq.T.astype(np.float64)
    kp = np.einsum("bsk,hk->bsh", keys, Wk, dtype=np.float64)
    sc = np.tanh(qp[:, None, :] + kp)
    en = np.einsum("bsh,h->bs", sc, v[0].astype(np.float64))
    en = np.where(mask, -np.inf, en)
    mx = np.max(en, axis=-1, keepdims=True)
    mx = np.where(np.isfinite(mx), mx, 0.0)
    ex = np.exp(en - mx)
    sm = ex.sum(axis=-1, keepdims=True)
    attn = np.where(sm > 0, ex / np.where(sm == 0, 1.0, sm), 0.0)
    ctxo = np.einsum("bs,bsk->bk", attn, keys, dtype=np.float64)
    return ctxo.astype(np.float32), attn.astype(np.float32)


_CACHE = {}


def _get_nc():
    if "nc" not in _CACHE:
        _CACHE["nc"] = build_kernel()
    return _CACHE["nc"]


def kernel(query, keys, mask, Wq, Wk, v):
    query = np.ascontiguousarray(np.asarray(query), dtype=np.float32)
    keys = np.ascontiguousarray(np.asarray(keys), dtype=np.float32)
    mask_u8 = np.ascontiguousarray(np.asarray(mask)).astype(np.uint8)
    Wq = np.ascontiguousarray(np.asarray(Wq), dtype=np.float32)
    Wk = np.ascontiguousarray(np.asarray(Wk), dtype=np.float32)
    v = np.ascontiguousarray(np.asarray(v), dtype=np.float32)

    nc = _get_nc()
    bs = FULL_B // N_CORES
    wqT, wkT, _, vT = pack_weights(Wq, Wk, v, query[0:bs])
    in_maps = []
    for c in range(N_CORES):
        sl = slice(c * bs, (c + 1) * bs)
        qT = pack_weights(Wq, Wk, v, query[sl])[2]
        in_maps.append(
            {
                "keys": np.ascontiguousarray(keys[sl]),
                "mask": np.ascontiguousarray(mask_u8[sl]),
                "wqT": wqT,
                "wkT": wkT,
                "qT": qT,
                "vT": vT,
            }
        )
    res = bass_utils.run_bass_kernel_spmd(nc, in_maps, core_ids=list(range(N_CORES)))
    context = np.concatenate([r["context"] for r in res.results], axis=0)
    attn = np.concatenate([r["attn"] for r in res.results], axis=0)
    return context, attn


# revision 74
# speedup vs baseline: 1.0041x; 1.0041x over previous
"""Bahdanau additive attention on TRN2 (Bass/Tile), 8-core data-parallel.

Math (per batch row b):
    qp   = query @ Wq.T                      # [H]
    kp_s = keys[s] @ Wk.T                    # [S, H]
    e_s  = v . tanh(qp + kp_s)               # [S]
    attn = softmax(mask(e))                  # [S]  (all-pad rows -> 0)
    ctx  = attn @ keys                       # [K]

Sharding: B=64 batches split 8 ways (8 per core); weights replicated.

Per-core kernel layout strategy:
  - The projection contracts over k, so keys tiles are PE-transposed
    ([s,k] -> [k,s]) and the matmul runs with Wk^T chunks stationary,
    producing kp^T [h, s] in PSUM (fp32r matmuls: full-rate with fp32
    storage).
  - tanh(kp^T + qp^T) is fused on ScalarE via activation bias (per
    partition = per h).
  - energy = v . tanh(...) is a second PE matmul contracting h.
  - softmax per batch row runs on partition 0 (tiny: [1, S]).
  - context re-streams keys in natural [s, k] layout and contracts s
    with the attention column as the stationary operand.
"""

import os
import sys

import numpy as np

for _p in ("/opt/trn_rl_repo", os.path.expanduser("~/.axon_site/_ro/trn_rl_repo")):
    if os.path.isdir(_p) and _p not in sys.path:
        sys.path.append(_p)

from contextlib import ExitStack

import concourse.bacc as bacc
import concourse.bass as bass
import concourse.mybir as mybir
import concourse.tile as tile
from concourse import bass_isa, bass_utils
from concourse.masks import make_identity

F32 = mybir.dt.float32
F32R = mybir.dt.float32r
BF16 = mybir.dt.bfloat16
U8 = mybir.dt.uint8
P = 128

N_CORES = 8
FULL_B, FULL_S, FULL_H, FULL_K = 64, 2048, 1024, 1024

NEG_BIG = -30.0  # masked-energy fill; |energy| << 30 so never collides


def _emit(ctx, tc, io, B, S, K, H, SB):
    nc = tc.nc
    KC, HC = K // P, H // P
    NB, JB, SC = SB // P, S // SB, S // P
    CTX_CH = [(o, min(512, K - o)) for o in range(0, K, 512)]
    TANH = mybir.ActivationFunctionType.Tanh
    EXP = mybir.ActivationFunctionType.Exp

    keys = io["keys"].ap()
    mask = io["mask"].ap()
    wqT_in = io["wqT"].ap()
    wkT_in = io["wkT"].ap()
    qT_in = io["qT"].ap()
    vT_in = io["vT"].ap()
    ctx_out = io["context"].ap()
    attn_out = io["attn"].ap()

    const = ctx.enter_context(tc.tile_pool(name="const", bufs=1))
    qpT = const.tile([P, HC, B], F32, name="qpT")

    # Host pre-packed weights: wkT/wqT are [P, KC, H] bf16 (W^T with the
    # contraction dim on partitions), qT is [P, KC, B] bf16, vT [P, HC]
    # bf16. Loaded with plain contiguous HWDGE DMAs; no on-chip transpose.
    # DMAs for these are emitted by load_weights() after the first keys
    # block so the kT pipeline owns the head of the sync queue.
    wkT = const.tile([P, KC, H], BF16, name="wkT")
    wqT = const.tile([P, KC, H], BF16, name="wqT")
    qT = const.tile([P, KC, B], BF16, name="qT")
    vTb = const.tile([P, HC], BF16, name="vTb")
    ident1 = const.tile([1, 1], F32, name="ident1")
    neg_big = const.tile([1, SB], F32, name="neg_big")

    def load_weights():
        nc.sync.dma_start(out=wkT, in_=wkT_in)
        nc.sync.dma_start(out=qT, in_=qT_in)
        nc.sync.dma_start(out=wqT, in_=wqT_in)
        nc.sync.dma_start(out=vTb, in_=vT_in)
        nc.vector.memset(ident1, 1.0)
        nc.vector.memset(neg_big, NEG_BIG)

    def wk_lhsT(hc, kc):
        return wkT[:, kc, hc * P : (hc + 1) * P]

    def qp_emit():
        for hc in range(HC):
            psqp = ps_kp.tile([P, B], F32, name="psqp", tag="kp")
            for kc in range(KC):
                nc.tensor.matmul(
                    psqp,
                    wqT[:, kc, hc * P : (hc + 1) * P],
                    qT[:, kc, :],
                    start=(kc == 0),
                    stop=(kc == KC - 1),
                )
            nc.vector.tensor_copy(qpT[:, hc, :], psqp)

    # ---------- main pools ----------
    knp = ctx.enter_context(tc.tile_pool(name="knp", bufs=2))
    knbp = ctx.enter_context(tc.tile_pool(name="knbp", bufs=7))
    ktp = ctx.enter_context(tc.tile_pool(name="ktp", bufs=3))
    thp = ctx.enter_context(tc.tile_pool(name="thp", bufs=9))
    stg = ctx.enter_context(tc.tile_pool(name="stg", bufs=2))
    atp = ctx.enter_context(tc.tile_pool(name="atp", bufs=2))
    PSUM = bass.MemorySpace.PSUM
    ps_kp = ctx.enter_context(tc.tile_pool(name="ps_kp", bufs=3, space=PSUM))
    ps_e = ctx.enter_context(tc.tile_pool(name="ps_e", bufs=2, space=PSUM))
    ps_c = ctx.enter_context(tc.tile_pool(name="ps_c", bufs=2, space=PSUM))

    def start_block(b, j):
        """HWDGE fp32 load -> DVE cast to bf16 -> x-bar DMA transpose."""
        kn = knp.tile([P, NB, K], F32, name="kn", tag="kn")
        nc.sync.dma_start(
            out=kn,
            in_=keys[b, j * SB : (j + 1) * SB, :].rearrange(
                "(nb p) k -> p nb k", p=P
            ),
        )
        knb = knbp.tile([P, NB, K], BF16, name="knb", tag="knb")
        nc.vector.tensor_copy(knb, kn)
        kT = ktp.tile([P, NB, KC, P], BF16, name="kT", tag="kT")
        nc.sync.dma_start_transpose(out=kT, in_=knb)
        return kT, knb

    # Energy matmuls are M=1; pack 4 of them into one PE pass on disjoint
    # 32-col groups (tile_position), giving 4 partial rows at partitions
    # 0/32/64/96 that GpSimd later all-reduces.
    EG = min(4, HC)  # energy col-pack width
    ER = (HC + EG - 1) // EG  # accumulation rounds per position

    def energy_pack(pe4, ths, r):
        for jj, (th, hc) in enumerate(ths):
            nc.tensor.matmul(
                pe4[32 * jj : 32 * jj + 1, :],
                vTb[:, hc : hc + 1],
                th,
                start=(r == 0),
                stop=(r == ER - 1),
                skip_group_check=True,
                tile_position=(0, 32 * jj),
            )

    def evac_energy(pe4, e_row4, b, j):
        sl = e_row4[0:1, j * SB : (j + 1) * SB]
        nc.scalar.copy(sl, pe4[0:1, :])
        for jj in range(1, EG):
            nc.vector.tensor_add(sl, sl, pe4[32 * jj : 32 * jj + 1, :])

    def compute_block(b, j, kT, e_row4, carry, post_first_group=None):
        pe4 = ps_e.tile([P, SB], F32, name="pe4", tag="e")
        ths = []
        r = 0
        for hc in range(HC):
            pk = ps_kp.tile([P, SB], F32, name="pk", tag="kp")
            for kc in range(KC):
                nc.tensor.matmul(
                    pk,
                    wk_lhsT(hc, kc),
                    kT[:, :, kc, :],
                    start=(kc == 0),
                    stop=(kc == KC - 1),
                )
            if post_first_group is not None:
                post_first_group()
                post_first_group = None
            th = thp.tile([P, SB], BF16, name="th", tag="th")
            nc.scalar.activation(th, pk, TANH, bias=qpT[:, hc, b : b + 1], scale=1.0)
            ths.append((th, hc))
            if carry is not None:
                # flush the previous packed group while tanh of this group
                # is still in flight on ScalarE
                flush_carry(carry)
                carry = None
            if len(ths) == EG:
                carry = (pe4, ths, r, (e_row4, b, j))
                ths = []
                r += 1
        return carry

    def flush_carry(carry):
        pe4, ths, r, evac_args = carry
        energy_pack(pe4, ths, r)
        if r == ER - 1:
            evac_energy(pe4, *evac_args)

    def softmax(b, e_row4, m_row):
        e_row = e_row4[0:1, :]
        for j in range(JB):
            nc.vector.copy_predicated(
                e_row[0:1, j * SB : (j + 1) * SB],
                m_row[0:1, j * SB : (j + 1) * SB],
                neg_big,
            )
        mx = stg.tile([1, 1], F32, name="mx", tag="s1")
        nc.vector.reduce_max(out=mx, in_=e_row, axis=mybir.AxisListType.X)
        nm = stg.tile([1, 1], F32, name="nm", tag="s2")
        nc.scalar.mul(nm, mx, -1.0)
        se = stg.tile([1, 1], F32, name="se", tag="s3")
        nc.scalar.activation(e_row, e_row, EXP, bias=nm, scale=1.0, accum_out=se)
        ri = stg.tile([1, 1], F32, name="ri", tag="s4")
        nc.vector.reciprocal(ri, se)
        # all-pad rows: max == NEG_BIG exactly -> zero the whole row
        pad = stg.tile([1, 1], F32, name="pad", tag="s5")
        nc.vector.tensor_scalar(
            out=pad, in0=nm, scalar1=-NEG_BIG, scalar2=None,
            op0=mybir.AluOpType.is_equal,
        )
        valid = stg.tile([1, 1], F32, name="valid", tag="s6")
        nc.vector.tensor_scalar(
            out=valid, in0=pad, scalar1=-1.0, scalar2=1.0,
            op0=mybir.AluOpType.mult, op1=mybir.AluOpType.add,
        )
        ri2 = stg.tile([1, 1], F32, name="ri2", tag="s7")
        nc.vector.tensor_mul(ri2, ri, valid)
        nc.vector.tensor_scalar_mul(e_row, e_row, ri2)
        nc.sync.dma_start(out=attn_out[b : b + 1, :], in_=e_row)
        return e_row

    def attn_transpose(b, a_row):
        at_ps = ps_e.tile([P, SC], F32, name="at_ps", tag="e")
        for sc in range(SC):
            nc.tensor.transpose(
                at_ps[:, sc : sc + 1],
                a_row[0:1, sc * P : (sc + 1) * P],
                ident1,
            )
        atT = atp.tile([P, SC], BF16, name="atT", tag="atT")
        nc.vector.tensor_copy(atT, at_ps)
        return atT

    CG = min(4, SC)  # context col-pack width
    CR = SC // CG

    def context_pass(b, atT, knbs_b):
        pc4s = [
            ps_c.tile([P, chn], F32, name=f"pc4_{i}", tag="c")
            for i, (o, chn) in enumerate(CTX_CH)
        ]
        for rr in range(CR):
            for i, (o, chn) in enumerate(CTX_CH):
                for jj in range(CG):
                    sc = rr * CG + jj
                    j2, nb = divmod(sc, NB)
                    nc.tensor.matmul(
                        pc4s[i][32 * jj : 32 * jj + 1, :],
                        atT[:, sc : sc + 1],
                        knbs_b[j2][:, nb, o : o + chn],
                        start=(rr == 0),
                        stop=(rr == CR - 1),
                        skip_group_check=True,
                        tile_position=(0, 32 * jj),
                    )
        ctx4 = stg.tile([1, K], F32, name="ctx4", tag="crow")
        for i, (o, chn) in enumerate(CTX_CH):
            sl = ctx4[0:1, o : o + chn]
            nc.scalar.copy(sl, pc4s[i][0:1, :])
            for jj in range(1, CG):
                nc.vector.tensor_add(sl, sl, pc4s[i][32 * jj : 32 * jj + 1, :])
        nc.sync.dma_start(out=ctx_out[b : b + 1, :], in_=ctx4[0:1, :])

    # ---------- main schedule ----------
    def new_b_tiles(b):
        e = stg.tile([1, S], F32, name="e_row4", tag="erow")
        m = stg.tile([1, S], U8, name="m_row", tag="mrow")
        nc.sync.dma_start(out=m, in_=mask[b : b + 1, :])
        return e, m

    blocks = [(b, j) for b in range(B) for j in range(JB)]
    LOOKAHEAD = 2
    e_rows, m_rows, a_rows = {}, {}, {}
    knbs, kts = {}, {}
    e_rows[0], m_rows[0] = new_b_tiles(0)
    load_weights()
    for i in range(min(LOOKAHEAD, len(blocks))):
        kts[blocks[i]], knbs[blocks[i]] = start_block(*blocks[i])
    qp_emit()
    carry = None
    for idx, (b, j) in enumerate(blocks):
        if idx + LOOKAHEAD < len(blocks):
            nxt_blk = blocks[idx + LOOKAHEAD]
            kts[nxt_blk], knbs[nxt_blk] = start_block(*nxt_blk)
        carry = compute_block(b, j, kts.pop((b, j)), e_rows[b], carry)
        if j == 0 and b > 0:
            context_pass(
                b - 1,
                attn_transpose(b - 1, a_rows[b - 1]),
                [knbs.pop((b - 1, jj)) for jj in range(JB)],
            )
        if j == JB - 1:
            if carry is not None:
                flush_carry(carry)
                carry = None
            a_rows[b] = softmax(b, e_rows[b], m_rows[b])
            if b + 1 < B:
                e_rows[b + 1], m_rows[b + 1] = new_b_tiles(b + 1)
    context_pass(
        B - 1,
        attn_transpose(B - 1, a_rows[B - 1]),
        [knbs.pop((B - 1, jj)) for jj in range(JB)],
    )


def build_kernel(B=FULL_B // N_CORES, S=FULL_S, K=FULL_K, H=FULL_H, SB=512):
    KC, HC = K // P, H // P
    nc = bacc.Bacc(
        "TRN2", target_bir_lowering=False, debug=False, enable_partition_id=False
    )
    io = {
        "keys": nc.dram_tensor("keys", [B, S, K], F32, kind="ExternalInput"),
        "mask": nc.dram_tensor("mask", [B, S], U8, kind="ExternalInput"),
        "wqT": nc.dram_tensor("wqT", [P, KC, H], BF16, kind="ExternalInput"),
        "wkT": nc.dram_tensor("wkT", [P, KC, H], BF16, kind="ExternalInput"),
        "qT": nc.dram_tensor("qT", [P, KC, B], BF16, kind="ExternalInput"),
        "vT": nc.dram_tensor("vT", [P, HC], BF16, kind="ExternalInput"),
        "context": nc.dram_tensor("context", [B, K], F32, kind="ExternalOutput"),
        "attn": nc.dram_tensor("attn", [B, S], F32, kind="ExternalOutput"),
    }
    with tile.TileContext(nc) as tc:
        with ExitStack() as ctx:
            _emit(ctx, tc, io, B, S, K, H, SB)
    nc.compile()
    return nc


def pack_weights(Wq, Wk, v, query):
    """Host-side re-layout: W^T/q^T/v^T in bf16 with the contraction dim
    on partitions. Pure marshalling (transpose + dtype cast), no math."""
    import ml_dtypes

    bf16 = ml_dtypes.bfloat16
    H, K = Wk.shape
    KC, HC = K // P, H // P
    Bq = query.shape[0]
    # wT[p, kc, h] = W[h, kc*P + p]
    wqT = np.ascontiguousarray(
        Wq.T.reshape(KC, P, H).transpose(1, 0, 2).astype(bf16)
    )
    wkT = np.ascontiguousarray(
        Wk.T.reshape(KC, P, H).transpose(1, 0, 2).astype(bf16)
    )
    # qT[p, kc, b] = query[b, kc*P + p]
    qT = np.ascontiguousarray(
        query.reshape(Bq, KC, P).transpose(2, 1, 0).astype(bf16)
    )
    # vT[p, hc] = v[0, hc*P + p]
    vT = np.ascontiguousarray(v[0].reshape(HC, P).T.astype(bf16))
    return wqT, wkT, qT, vT


def ref_np(query, keys, mask, Wq, Wk, v):
    """Numpy reference (mirrors the jax oracle) for dev testing."""
    qp = query.astype(np.float64) @ Wq.T.astype(np.float64)
    kp = np.einsum("bsk,hk->bsh", keys, Wk, dtype=np.float64)
    sc = np.tanh(qp[:, None, :] + kp)
    en = np.einsum("bsh,h->bs", sc, v[0].astype(np.float64))
    en = np.where(mask, -np.inf, en)
    mx = np.max(en, axis=-1, keepdims=True)
    mx = np.where(np.isfinite(mx), mx, 0.0)
    ex = np.exp(en - mx)
    sm = ex.sum(axis=-1, keepdims=True)
    attn = np.where(sm > 0, ex / np.where(sm == 0, 1.0, sm), 0.0)
    ctxo = np.einsum("bs,bsk->bk", attn, keys, dtype=np.float64)
    return ctxo.astype(np.float32), attn.astype(np.float32)


_CACHE = {}


def _get_nc():
    if "nc" not in _CACHE:
        _CACHE["nc"] = build_kernel()
    return _CACHE["nc"]


def kernel(query, keys, mask, Wq, Wk, v):
    query = np.ascontiguousarray(np.asarray(query), dtype=np.float32)
    keys = np.ascontiguousarray(np.asarray(keys), dtype=np.float32)
    mask_u8 = np.ascontiguousarray(np.asarray(mask)).astype(np.uint8)
    Wq = np.ascontiguousarray(np.asarray(Wq), dtype=np.float32)
    Wk = np.ascontiguousarray(np.asarray(Wk), dtype=np.float32)
    v = np.ascontiguousarray(np.asarray(v), dtype=np.float32)

    nc = _get_nc()
    bs = FULL_B // N_CORES
    wqT, wkT, _, vT = pack_weights(Wq, Wk, v, query[0:bs])
    in_maps = []
    for c in range(N_CORES):
        sl = slice(c * bs, (c + 1) * bs)
        qT = pack_weights(Wq, Wk, v, query[sl])[2]
        in_maps.append(
            {
                "keys": np.ascontiguousarray(keys[sl]),
                "mask": np.ascontiguousarray(mask_u8[sl]),
                "wqT": wqT,
                "wkT": wkT,
                "qT": qT,
                "vT": vT,
            }
        )
    res = bass_utils.run_bass_kernel_spmd(nc, in_maps, core_ids=list(range(N_CORES)))
    context = np.concatenate([r["context"] for r in res.results], axis=0)
    attn = np.concatenate([r["attn"] for r in res.results], axis=0)
    return context, attn


# revision 76
# speedup vs baseline: 1.0330x; 1.0288x over previous
"""Bahdanau additive attention on TRN2 (Bass/Tile), 8-core data-parallel.

Math (per batch row b):
    qp   = query @ Wq.T                      # [H]
    kp_s = keys[s] @ Wk.T                    # [S, H]
    e_s  = v . tanh(qp + kp_s)               # [S]
    attn = softmax(mask(e))                  # [S]  (all-pad rows -> 0)
    ctx  = attn @ keys                       # [K]

Sharding: B=64 batches split 8 ways (8 per core); weights replicated.

Per-core kernel layout strategy:
  - The projection contracts over k, so keys tiles are PE-transposed
    ([s,k] -> [k,s]) and the matmul runs with Wk^T chunks stationary,
    producing kp^T [h, s] in PSUM (fp32r matmuls: full-rate with fp32
    storage).
  - tanh(kp^T + qp^T) is fused on ScalarE via activation bias (per
    partition = per h).
  - energy = v . tanh(...) is a second PE matmul contracting h.
  - softmax per batch row runs on partition 0 (tiny: [1, S]).
  - context re-streams keys in natural [s, k] layout and contracts s
    with the attention column as the stationary operand.
"""

import os
import sys

import numpy as np

for _p in ("/opt/trn_rl_repo", os.path.expanduser("~/.axon_site/_ro/trn_rl_repo")):
    if os.path.isdir(_p) and _p not in sys.path:
        sys.path.append(_p)

from contextlib import ExitStack

import concourse.bacc as bacc
import concourse.bass as bass
import concourse.mybir as mybir
import concourse.tile as tile
from concourse import bass_isa, bass_utils
from concourse.masks import make_identity

F32 = mybir.dt.float32
F32R = mybir.dt.float32r
BF16 = mybir.dt.bfloat16
U8 = mybir.dt.uint8
P = 128

N_CORES = 8
FULL_B, FULL_S, FULL_H, FULL_K = 64, 2048, 1024, 1024

NEG_BIG = -30.0  # masked-energy fill; |energy| << 30 so never collides


def _emit(ctx, tc, io, B, S, K, H, SB):
    nc = tc.nc
    KC, HC = K // P, H // P
    NB, JB, SC = SB // P, S // SB, S // P
    CTX_CH = [(o, min(512, K - o)) for o in range(0, K, 512)]
    TANH = mybir.ActivationFunctionType.Tanh
    EXP = mybir.ActivationFunctionType.Exp

    keys = io["keys"].ap()
    mask = io["mask"].ap()
    wqT_in = io["wqT"].ap()
    wkT_in = io["wkT"].ap()
    qT_in = io["qT"].ap()
    vT_in = io["vT"].ap()
    ctx_out = io["context"].ap()
    attn_out = io["attn"].ap()

    const = ctx.enter_context(tc.tile_pool(name="const", bufs=1))
    qpT = const.tile([P, HC, B], F32, name="qpT")

    # Host pre-packed weights: wkT/wqT are [P, KC, H] bf16 (W^T with the
    # contraction dim on partitions), qT is [P, KC, B] bf16, vT [P, HC]
    # bf16. Loaded with plain contiguous HWDGE DMAs; no on-chip transpose.
    # DMAs for these are emitted by load_weights() after the first keys
    # block so the kT pipeline owns the head of the sync queue.
    wkT = const.tile([P, KC, H], BF16, name="wkT")
    wqT = const.tile([P, KC, H], BF16, name="wqT")
    qT = const.tile([P, KC, B], BF16, name="qT")
    vTb = const.tile([P, HC], BF16, name="vTb")
    ident1 = const.tile([1, 1], F32, name="ident1")
    neg_big = const.tile([1, SB], F32, name="neg_big")

    def load_weights():
        nc.sync.dma_start(out=wkT, in_=wkT_in)
        nc.sync.dma_start(out=qT, in_=qT_in)
        nc.sync.dma_start(out=wqT, in_=wqT_in)
        nc.sync.dma_start(out=vTb, in_=vT_in)
        nc.vector.memset(ident1, 1.0)
        nc.vector.memset(neg_big, NEG_BIG)

    def wk_lhsT(hc, kc):
        return wkT[:, kc, hc * P : (hc + 1) * P]

    def qp_emit():
        for hc in range(HC):
            psqp = ps_kp.tile([P, B], F32, name="psqp", tag="kp")
            for kc in range(KC):
                nc.tensor.matmul(
                    psqp,
                    wqT[:, kc, hc * P : (hc + 1) * P],
                    qT[:, kc, :],
                    start=(kc == 0),
                    stop=(kc == KC - 1),
                )
            nc.vector.tensor_copy(qpT[:, hc, :], psqp)

    # ---------- main pools ----------
    knp = ctx.enter_context(tc.tile_pool(name="knp", bufs=2))
    knbp = ctx.enter_context(tc.tile_pool(name="knbp", bufs=7))
    ktp = ctx.enter_context(tc.tile_pool(name="ktp", bufs=3))
    thp = ctx.enter_context(tc.tile_pool(name="thp", bufs=9))
    stg = ctx.enter_context(tc.tile_pool(name="stg", bufs=2))
    atp = ctx.enter_context(tc.tile_pool(name="atp", bufs=2))
    PSUM = bass.MemorySpace.PSUM
    ps_kp = ctx.enter_context(tc.tile_pool(name="ps_kp", bufs=3, space=PSUM))
    ps_e = ctx.enter_context(tc.tile_pool(name="ps_e", bufs=2, space=PSUM))
    ps_c = ctx.enter_context(tc.tile_pool(name="ps_c", bufs=2, space=PSUM))

    def start_block(b, j):
        """HWDGE fp32 load -> DVE cast to bf16 -> x-bar DMA transpose."""
        kn = knp.tile([P, NB, K], F32, name="kn", tag="kn")
        nc.sync.dma_start(
            out=kn,
            in_=keys[b, j * SB : (j + 1) * SB, :].rearrange(
                "(nb p) k -> p nb k", p=P
            ),
        )
        knb = knbp.tile([P, NB, K], BF16, name="knb", tag="knb")
        nc.vector.tensor_copy(knb, kn)
        kT = ktp.tile([P, NB, KC, P], BF16, name="kT", tag="kT")
        nc.sync.dma_start_transpose(out=kT, in_=knb)
        return kT, knb

    # Energy matmuls are M=1; pack 4 of them into one PE pass on disjoint
    # 32-col groups (tile_position), giving 4 partial rows at partitions
    # 0/32/64/96 that GpSimd later all-reduces.
    EG = min(4, HC)  # energy col-pack width
    ER = (HC + EG - 1) // EG  # accumulation rounds per position

    def energy_pack(pe4, ths, r):
        for jj, (th, hc) in enumerate(ths):
            nc.tensor.matmul(
                pe4[32 * jj : 32 * jj + 1, :],
                vTb[:, hc : hc + 1],
                th,
                start=(r == 0),
                stop=(r == ER - 1),
                skip_group_check=True,
                tile_position=(0, 32 * jj),
            )

    def evac_energy(pe4, e_row4, b, j):
        sl = e_row4[0:1, j * SB : (j + 1) * SB]
        nc.scalar.copy(sl, pe4[0:1, :])
        for jj in range(1, EG):
            nc.vector.tensor_add(sl, sl, pe4[32 * jj : 32 * jj + 1, :])

    def compute_block(b, j, kT, e_row4, carry, post_first_group=None):
        pe4 = ps_e.tile([P, SB], F32, name="pe4", tag="e")
        ths = []
        r = 0
        for hc in range(HC):
            pk = ps_kp.tile([P, SB], F32, name="pk", tag="kp")
            for kc in range(KC):
                nc.tensor.matmul(
                    pk,
                    wk_lhsT(hc, kc),
                    kT[:, :, kc, :],
                    start=(kc == 0),
                    stop=(kc == KC - 1),
                )
            if post_first_group is not None:
                post_first_group()
                post_first_group = None
            th = thp.tile([P, SB], BF16, name="th", tag="th")
            nc.scalar.activation(th, pk, TANH, bias=qpT[:, hc, b : b + 1], scale=1.0)
            ths.append((th, hc))
            if carry is not None:
                # flush the previous packed group while tanh of this group
                # is still in flight on ScalarE
                flush_carry(carry)
                carry = None
            if len(ths) == EG:
                carry = (pe4, ths, r, (e_row4, b, j))
                ths = []
                r += 1
        return carry

    def flush_carry(carry):
        pe4, ths, r, evac_args = carry
        energy_pack(pe4, ths, r)
        if r == ER - 1:
            evac_energy(pe4, *evac_args)

    def softmax(b, e_row4, m_row):
        e_row = e_row4[0:1, :]
        for j in range(JB):
            nc.vector.copy_predicated(
                e_row[0:1, j * SB : (j + 1) * SB],
                m_row[0:1, j * SB : (j + 1) * SB],
                neg_big,
            )
        mx = stg.tile([1, 1], F32, name="mx", tag="s1")
        nc.vector.reduce_max(out=mx, in_=e_row, axis=mybir.AxisListType.X)
        nm = stg.tile([1, 1], F32, name="nm", tag="s2")
        nc.scalar.mul(nm, mx, -1.0)
        se = stg.tile([1, 1], F32, name="se", tag="s3")
        nc.scalar.activation(e_row, e_row, EXP, bias=nm, scale=1.0, accum_out=se)
        ri = stg.tile([1, 1], F32, name="ri", tag="s4")
        nc.vector.reciprocal(ri, se)
        # all-pad rows: max == NEG_BIG exactly -> zero the whole row
        pad = stg.tile([1, 1], F32, name="pad", tag="s5")
        nc.vector.tensor_scalar(
            out=pad, in0=nm, scalar1=-NEG_BIG, scalar2=None,
            op0=mybir.AluOpType.is_equal,
        )
        valid = stg.tile([1, 1], F32, name="valid", tag="s6")
        nc.vector.tensor_scalar(
            out=valid, in0=pad, scalar1=-1.0, scalar2=1.0,
            op0=mybir.AluOpType.mult, op1=mybir.AluOpType.add,
        )
        ri2 = stg.tile([1, 1], F32, name="ri2", tag="s7")
        nc.vector.tensor_mul(ri2, ri, valid)
        nc.vector.tensor_scalar_mul(e_row, e_row, ri2)
        nc.sync.dma_start(out=attn_out[b : b + 1, :], in_=e_row)
        return e_row

    def attn_transpose(b, a_row):
        at_ps = ps_e.tile([P, SC], F32, name="at_ps", tag="e")
        for sc in range(SC):
            nc.tensor.transpose(
                at_ps[:, sc : sc + 1],
                a_row[0:1, sc * P : (sc + 1) * P],
                ident1,
            )
        atT = atp.tile([P, SC], BF16, name="atT", tag="atT")
        nc.vector.tensor_copy(atT, at_ps)
        return atT

    CG = min(4, SC)  # context col-pack width
    CR = SC // CG

    def context_pass(b, atT, knbs_b):
        pc4s = [
            ps_c.tile([P, chn], F32, name=f"pc4_{i}", tag="c")
            for i, (o, chn) in enumerate(CTX_CH)
        ]
        for rr in range(CR):
            for i, (o, chn) in enumerate(CTX_CH):
                for jj in range(CG):
                    sc = rr * CG + jj
                    j2, nb = divmod(sc, NB)
                    nc.tensor.matmul(
                        pc4s[i][32 * jj : 32 * jj + 1, :],
                        atT[:, sc : sc + 1],
                        knbs_b[j2][:, nb, o : o + chn],
                        start=(rr == 0),
                        stop=(rr == CR - 1),
                        skip_group_check=True,
                        tile_position=(0, 32 * jj),
                    )
        ctx4 = stg.tile([1, K], F32, name="ctx4", tag="crow")
        for i, (o, chn) in enumerate(CTX_CH):
            sl = ctx4[0:1, o : o + chn]
            nc.scalar.copy(sl, pc4s[i][0:1, :])
            for jj in range(1, CG):
                nc.vector.tensor_add(sl, sl, pc4s[i][32 * jj : 32 * jj + 1, :])
        nc.sync.dma_start(out=ctx_out[b : b + 1, :], in_=ctx4[0:1, :])

    # ---------- main schedule ----------
    def new_b_tiles(b):
        e = stg.tile([1, S], F32, name="e_row4", tag="erow")
        m = stg.tile([1, S], U8, name="m_row", tag="mrow")
        nc.sync.dma_start(out=m, in_=mask[b : b + 1, :])
        return e, m

    blocks = [(b, j) for b in range(B) for j in range(JB)]
    LOOKAHEAD = 2
    e_rows, m_rows, a_rows = {}, {}, {}
    knbs, kts = {}, {}
    e_rows[0], m_rows[0] = new_b_tiles(0)
    load_weights()
    for i in range(min(LOOKAHEAD, len(blocks))):
        kts[blocks[i]], knbs[blocks[i]] = start_block(*blocks[i])
    qp_emit()
    carry = None
    for idx, (b, j) in enumerate(blocks):
        if idx + LOOKAHEAD < len(blocks):
            nxt_blk = blocks[idx + LOOKAHEAD]
            kts[nxt_blk], knbs[nxt_blk] = start_block(*nxt_blk)
        carry = compute_block(b, j, kts.pop((b, j)), e_rows[b], carry)
        # Row b-1's final energy pack was flushed inside the compute
        # block above (after its first projection group, when the tanh it
        # needs has long retired) — only now is e_row(b-1) complete.
        if j == 0 and b > 0:
            a_rows[b - 1] = softmax(b - 1, e_rows[b - 1], m_rows[b - 1])
            context_pass(
                b - 1,
                attn_transpose(b - 1, a_rows[b - 1]),
                [knbs.pop((b - 1, jj)) for jj in range(JB)],
            )
        if j == JB - 1 and b + 1 < B:
            e_rows[b + 1], m_rows[b + 1] = new_b_tiles(b + 1)
    if carry is not None:
        flush_carry(carry)
    a_rows[B - 1] = softmax(B - 1, e_rows[B - 1], m_rows[B - 1])
    context_pass(
        B - 1,
        attn_transpose(B - 1, a_rows[B - 1]),
        [knbs.pop((B - 1, jj)) for jj in range(JB)],
    )


def build_kernel(B=FULL_B // N_CORES, S=FULL_S, K=FULL_K, H=FULL_H, SB=512):
    KC, HC = K // P, H // P
    nc = bacc.Bacc(
        "TRN2", target_bir_lowering=False, debug=False, enable_partition_id=False
    )
    io = {
        "keys": nc.dram_tensor("keys", [B, S, K], F32, kind="ExternalInput"),
        "mask": nc.dram_tensor("mask", [B, S], U8, kind="ExternalInput"),
        "wqT": nc.dram_tensor("wqT", [P, KC, H], BF16, kind="ExternalInput"),
        "wkT": nc.dram_tensor("wkT", [P, KC, H], BF16, kind="ExternalInput"),
        "qT": nc.dram_tensor("qT", [P, KC, B], BF16, kind="ExternalInput"),
        "vT": nc.dram_tensor("vT", [P, HC], BF16, kind="ExternalInput"),
        "context": nc.dram_tensor("context", [B, K], F32, kind="ExternalOutput"),
        "attn": nc.dram_tensor("attn", [B, S], F32, kind="ExternalOutput"),
    }
    with tile.TileContext(nc) as tc:
        with ExitStack() as ctx:
            _emit(ctx, tc, io, B, S, K, H, SB)
    nc.compile()
    return nc


def pack_weights(Wq, Wk, v, query):
    """Host-side re-layout: W^T/q^T/v^T in bf16 with the contraction dim
    on partitions. Pure marshalling (transpose + dtype cast), no math."""
    import ml_dtypes

    bf16 = ml_dtypes.bfloat16
    H, K = Wk.shape
    KC, HC = K // P, H // P
    Bq = query.shape[0]
    # wT[p, kc, h] = W[h, kc*P + p]
    wqT = np.ascontiguousarray(
        Wq.T.reshape(KC, P, H).transpose(1, 0, 2).astype(bf16)
    )
    wkT = np.ascontiguousarray(
        Wk.T.reshape(KC, P, H).transpose(1, 0, 2).astype(bf16)
    )
    # qT[p, kc, b] = query[b, kc*P + p]
    qT = np.ascontiguousarray(
        query.reshape(Bq, KC, P).transpose(2, 1, 0).astype(bf16)
    )
    # vT[p, hc] = v[0, hc*P + p]
    vT = np.ascontiguousarray(v[0].reshape(HC, P).T.astype(bf16))
    return wqT, wkT, qT, vT


def ref_np(query, keys, mask, Wq, Wk, v):
    """Numpy reference (mirrors the jax oracle) for dev testing."""
    qp = query.astype(np.float64) @ Wq.T.astype(np.float64)
    kp = np.einsum("bsk,hk->bsh", keys, Wk, dtype=np.float64)
    sc = np.tanh(qp[:, None, :] + kp)
    en = np.einsum("bsh,h->bs", sc, v[0].astype(np.float64))
    en = np.where(mask, -np.inf, en)
    mx = np.max(en, axis=-1, keepdims=True)
    mx = np.where(np.isfinite(mx), mx, 0.0)
    ex = np.exp(en - mx)
    sm = ex.sum(axis=-1, keepdims=True)
    attn = np.where(sm > 0, ex / np.where(sm == 0, 1.0, sm), 0.0)
    ctxo = np.einsum("bs,bsk->bk", attn, keys, dtype=np.float64)
    return ctxo.astype(np.float32), attn.astype(np.float32)


_CACHE = {}


def _get_nc():
    if "nc" not in _CACHE:
        _CACHE["nc"] = build_kernel()
    return _CACHE["nc"]


def kernel(query, keys, mask, Wq, Wk, v):
    query = np.ascontiguousarray(np.asarray(query), dtype=np.float32)
    keys = np.ascontiguousarray(np.asarray(keys), dtype=np.float32)
    mask_u8 = np.ascontiguousarray(np.asarray(mask)).astype(np.uint8)
    Wq = np.ascontiguousarray(np.asarray(Wq), dtype=np.float32)
    Wk = np.ascontiguousarray(np.asarray(Wk), dtype=np.float32)
    v = np.ascontiguousarray(np.asarray(v), dtype=np.float32)

    nc = _get_nc()
    bs = FULL_B // N_CORES
    wqT, wkT, _, vT = pack_weights(Wq, Wk, v, query[0:bs])
    in_maps = []
    for c in range(N_CORES):
        sl = slice(c * bs, (c + 1) * bs)
        qT = pack_weights(Wq, Wk, v, query[sl])[2]
        in_maps.append(
            {
                "keys": np.ascontiguousarray(keys[sl]),
                "mask": np.ascontiguousarray(mask_u8[sl]),
                "wqT": wqT,
                "wkT": wkT,
                "qT": qT,
                "vT": vT,
            }
        )
    res = bass_utils.run_bass_kernel_spmd(nc, in_maps, core_ids=list(range(N_CORES)))
    context = np.concatenate([r["context"] for r in res.results], axis=0)
    attn = np.concatenate([r["attn"] for r in res.results], axis=0)
    return context, attn
